# revision 1
# baseline (speedup 1.0000x reference)
"""Trainium2 Bass kernel for the Hoyer-spike attention module (B=8,N=1024,C=768,H=12).

Math (per batch, per head): xf = spike1(x); [q|k|v] = xf @ qkv_w.T; ks,vs =
spike2(k),spike2(v) (binary); y = q @ (ks.T @ vs) (exact reassociation of
(q@ks.T)@vs -- no softmax); z = spike3(y) with torch's reshape(B,H,D,N)
reinterpretation; out = z @ proj_w.T + proj_b.

Distribution: data-parallel over B=8 -> one batch per NeuronCore, weights
replicated, no collectives.

Numerics: the q/k/v matmuls feed binary spikes, so any relative error eps
in the pre-spike values flips ~eps of the bits and costs ~sqrt(eps) in
final relative error -- they need ~16-bit-clean weights. HW-measured
facts that drive the format choices: fp16 matmuls are exact (~3e-7),
fp32r carries ~1e-4 internal noise (too big once spike-amplified), and
fp8 DoubleRow noise scales with its own pass's magnitude. So q/kv
weights are split fp16-hi (1 cycle/row, 6 chunk matmuls) + e5m2-lo
scaled x128 with a 1/128-valued xf copy (DoubleRow, 0.5 cycles/row, 3
pair matmuls) = 1.5 cycles/row at ~1e-5 effective weight error, vs 2.0
for bf16 hi+lo. M = ks.T@vs is exact integers <=1024 (fp16-exact); the
y-matmul packs q-hi and q-lo fp16 into the two partition halves of one
128-deep contraction against M duplicated on both halves -- hi+lo in a
single matmul per (head, n-block). proj runs fp8e4 hi+lo DoubleRow
(x1024 row scale; post-spike, so fp8-level error stays direct, not
amplified); z blocks from even head-pairs emit Sign in {-1,+1} on the
Activation engine and odd blocks emit {0,1} on DVE, with the encoding
difference folded into per-row proj weight scales and a colsum term in
the per-partition output bias. BN/Hoyer affines fold host-side (A_o
into the q weight columns) so every post-matmul op is one instruction.

Engines: spikes on DVE (z split DVE/Act); qT-hi copy, xf/128 copy, M
copy and the proj epilogue (scale + bias) on the Activation engine;
partition-moving copies (q-lo packing, M duplication) as SBUF->SBUF
DMAs on otherwise-idle queues.

Layouts: x and weights host-transposed; xf serves as stationary operand
(k|v) and moving operand (qT). The torch reshape(B,H,D,N) shuffle is a
pure addressing trick: the y-matmul's stationary q slices use a
stride-16 access pattern so its PSUM output lands directly in zT layout
for the proj matmul; head pairs are packed via tile_position row+col
quadrants (HW-validated construct). The proj matmul runs transposed
(weights stationary, z moving) so proj_b becomes a per-partition
Activation bias and the output DMAs contiguously from a [C, N] DRAM
buffer (host transposes back). Emission is software-pipelined: P2 runs
in two 6-psum waves interleaved chunk-outer with the x/wq DMA stream,
and each chunk's M-matmuls trail two chunks behind their kv spikes.
"""
import sys
sys.path.insert(0, '/opt/trn_rl_repo')
import numpy as np
import ml_dtypes

import concourse.bass as bass
import concourse.mybir as mybir
import concourse.tile as tile
from concourse import bacc

F32 = mybir.dt.float32
F32R = mybir.dt.float32r
BF16 = mybir.dt.bfloat16
FP16 = mybir.dt.float16
FP8 = mybir.dt.float8e4
FP8E5 = mybir.dt.float8e5
AOT = mybir.AluOpType
DR = mybir.MatmulPerfMode.DoubleRow
ACT = mybir.ActivationFunctionType

B, N, C, H, D = 8, 1024, 768, 12, 64
EPS, XS = 1e-5, 1.0
NCORES = 8
E4 = np.dtype(ml_dtypes.float8_e4m3)
BFD = np.dtype(ml_dtypes.bfloat16)
E5 = np.dtype(ml_dtypes.float8_e5m2)
SP = 1024.0   # proj weight scale; z spike value 1/64 -> psum = SP/64 * out


def build_nc(rounds=1, upto=5):
    nc = bacc.Bacc(None, target_bir_lowering=False)
    xt_d = nc.declare_dram_parameter("xt", [C, N], F32, isOutput=False)
    wqh_d = nc.declare_dram_parameter("wqh", [C, C], FP16, isOutput=False)
    wql_d = nc.declare_dram_parameter("wql", [C, C], FP8E5, isOutput=False)
    wkvh_d = nc.declare_dram_parameter("wkvh", [C, 2 * C], FP16, isOutput=False)
    wkvl_d = nc.declare_dram_parameter("wkvl", [C, 2 * C], FP8E5, isOutput=False)
    phi_d = nc.declare_dram_parameter("p_hi", [C, C], FP8, isOutput=False)
    plo_d = nc.declare_dram_parameter("p_lo", [C, C], FP8, isOutput=False)
    pb_d = nc.declare_dram_parameter("pb", [128, 6], F32, isOutput=False)
    txa_d = nc.declare_dram_parameter("txA", [128, 6], F32, isOutput=False)
    txt_d = nc.declare_dram_parameter("txT", [128, 6], F32, isOutput=False)
    tkv_d = nc.declare_dram_parameter("tkv", [128, 2 * C], F32, isOutput=False)
    tyt_d = nc.declare_dram_parameter("tyT", [128, 6], F32, isOutput=False)
    tytn_d = nc.declare_dram_parameter("tytn", [128, 6], F32, isOutput=False)
    out_d = nc.declare_dram_parameter("out", [C, N], F32, isOutput=True)

    with tile.TileContext(nc) as tc:
        with (
            tc.tile_pool(name="const", bufs=1) as const,
            tc.tile_pool(name="work", bufs=2) as work,
            tc.tile_pool(name="mm", bufs=7, space="PSUM") as mm,
            tc.tile_pool(name="mps", bufs=1, space="PSUM") as mps,
        ):
            # ---- constants ----
            txa = const.tile([128, 6], F32, name="txa")
            txt = const.tile([128, 6], F32, name="txt")
            tkv = const.tile([128, 2 * C], F32, name="tkv")
            tyt = const.tile([128, 6], F32, name="tyt")
            tytn = const.tile([128, 6], F32, name="tytn")
            nc.sync.dma_start(txa[:], txa_d[:])
            nc.sync.dma_start(txt[:], txt_d[:])

            wqh = const.tile([128, 6 * C], FP16, name="wqh")
            wql = const.tile([128, 6 * C], FP8E5, name="wql")
            wkvh = const.tile([128, 6 * 2 * C], FP16, name="wkvh")
            wkvl = const.tile([128, 6 * 2 * C], FP8E5, name="wkvl")
            p_hi = const.tile([128, 6 * C], FP8, name="p_hi")
            p_lo = const.tile([128, 6 * C], FP8, name="p_lo")
            pb = const.tile([128, 6], F32, name="pb")

            phi3 = p_hi[:, :].rearrange("p (t c) -> p t c", t=6)
            plo3 = p_lo[:, :].rearrange("p (t c) -> p t c", t=6)

            for _r in range(rounds):
                # ---- phase 1: xT -> spike -> xf (f32 binary) ----
                # DMA order follows first use: x chunks + q-weight chunks
                # first (phase 2 can start), then k|v weights, proj last.
                xf = const.tile([128, 6 * N], FP16, name=f"xf_{_r}", tag="xf")
                xfl = const.tile([128, 6 * N], FP8E5, name=f"xfl_{_r}", tag="xfl")
                xtss = []
                for ck in range(6):
                    xts = work.tile([128, N], F32, name=f"xts{ck}_{_r}", tag="xt")
                    xtss.append(xts)
                    # half-chunk granularity so the PE consumes x as it
                    # lands; wqh alternates between the SP and Activation DMA
                    # queues so the weight stream paces at 2x
                    pieces = ([(0, 256), (256, 512), (512, 1024)] if ck == 0
                              else [(0, 512), (512, 1024)])
                    for lo_, hi_ in pieces:
                        hs = slice(lo_, hi_)
                        nc.gpsimd.dma_start(xts[:, hs],
                                            xt_d[ck * 128:(ck + 1) * 128, hs])
                        nc.vector.tensor_scalar(xf[:, ck * N + lo_: ck * N + hi_],
                                                xts[:, hs],
                                                txa[:, ck:ck + 1], txt[:, ck:ck + 1],
                                                AOT.mult, AOT.is_ge)
                        nc.scalar.activation(xfl[:, ck * N + lo_: ck * N + hi_],
                                             xf[:, ck * N + lo_: ck * N + hi_],
                                             ACT.Identity, bias=0.0, scale=1.0 / 128)
                    q_dma = nc.sync.dma_start if ck % 2 == 0 else nc.scalar.dma_start
                    q_dma(wqh[:, ck * C:(ck + 1) * C],
                          wqh_d[ck * 128:(ck + 1) * 128, :])
                    nc.sync.dma_start(wql[:, ck * C:(ck + 1) * C],
                                      wql_d[ck * 128:(ck + 1) * 128, :])
                nc.sync.dma_start(tkv[:], tkv_d[:])
                nc.sync.dma_start(tyt[:], tyt_d[:])
                nc.sync.dma_start(tytn[:], tytn_d[:])
                for ck in range(6):
                    nc.sync.dma_start(wkvh[:, ck * 2 * C:(ck + 1) * 2 * C],
                                      wkvh_d[ck * 128:(ck + 1) * 128, :])
                for ck in range(6):
                    nc.sync.dma_start(wkvl[:, ck * 2 * C:(ck + 1) * 2 * C],
                                      wkvl_d[ck * 128:(ck + 1) * 128, :])
                nc.sync.dma_start(pb[:], pb_d[:])
                xfl3 = xfl[:, :].rearrange("p (t n) -> p t n", t=6)
                wql3 = wql[:, :].rearrange("p (t c) -> p t c", t=6)
                wkvl3 = wkvl[:, :].rearrange("p (t c) -> p t c", t=6)
                for ck in range(6):
                    nc.sync.dma_start(p_hi[:, ck * C:(ck + 1) * C],
                                      phi_d[ck * 128:(ck + 1) * 128, :])
                    nc.sync.dma_start(p_lo[:, ck * C:(ck + 1) * C],
                                      plo_d[ck * 128:(ck + 1) * 128, :])

                if upto < 2:
                    nc.sync.dma_start(out_d[0:128, 0:N], xtss[0][:, 0:N])
                    continue
                # ---- phase 2: qT (shuffled layout) = A_o * (Wq @ xfT), fp16
                # hi+lo split so the y-matmul runs exact fp16 passes ----
                # chunk hp holds heads (2hp, 2hp+1) on partitions 0:64 / 64:128.
                # Shuffled free axis: col m = (n%16)*64 + n//16 so the y-matmul
                # lhsT slices are contiguous.
                # per-head packed q: partitions 0:64 = hi, 64:128 = lo for
                # even heads; reversed for odd heads; the y-matmul contracts
                # all 128 partitions to get hi+lo in one pass (m16 holds M on
                # both halves)
                qTh = [const.tile([128, N], FP16, name=f"qTh{hp}_{_r}", tag=f"qTh{hp}")
                       for hp in range(6)]
                qTl = [const.tile([128, N], FP16, name=f"qTl{hp}_{_r}", tag=f"qTl{hp}")
                       for hp in range(6)]
                qp = [const.tile([128, N], FP16, name=f"qp{h}_{_r}", tag=f"qp{h}")
                      for h in range(H)]
                # waves of 4 psums, chunk-outer inside each wave so the PE
                # consumes x/wq chunks as their DMAs land
                kv_done = set()
                kvss = [None] * 8

                def emit_kv_psum(nk, kvf):
                    kvs = kvss[nk]
                    p = mm.tile([128, 512], F32, name=f"kvp{nk}_{kvf}_{_r}", tag="mm")
                    for ck in range(6):
                        nc.tensor.matmul(
                            p[:],
                            xf[:, ck * N + nk * 128: ck * N + (nk + 1) * 128],
                            wkvh[:, ck * 2 * C + kvf * 512: ck * 2 * C + (kvf + 1) * 512],
                            start=(ck == 0), stop=False)
                    for tp in range(3):
                        nc.tensor.matmul(
                            p[:],
                            xfl3[:, 2 * tp:2 * tp + 2, nk * 128:(nk + 1) * 128],
                            wkvl3[:, 2 * tp:2 * tp + 2, kvf * 512:(kvf + 1) * 512],
                            start=False, stop=(tp == 2), perf_mode=DR)
                    nc.vector.tensor_tensor(
                        kvs[:, kvf * 512:(kvf + 1) * 512], p[:],
                        tkv[:, kvf * 512:(kvf + 1) * 512], AOT.is_ge)
                for wave in range(2):
                    pairs = [(wave * 3 + hp, nf) for hp in range(3) for nf in range(2)]
                    ps = {}
                    for pr in pairs:
                        if wave == 0 and pr == (0, 0):
                            # the very first psum is split into two
                            # independent 256-wide groups fed by a quartered
                            # first x piece, so the PE starts ~1.5us earlier
                            ps[pr] = [mm.tile([128, 256], F32,
                                              name=f"qp00{j}_{_r}", tag="mm")
                                      for j in range(2)]
                        else:
                            ps[pr] = mm.tile([128, 512], F32,
                                             name=f"qp{pr[0]}_{pr[1]}_{_r}", tag="mm")
                    for ck in range(6):
                        for hp, nf in pairs:
                            w_sl = wqh[:, ck * C + hp * 128: ck * C + (hp + 1) * 128]
                            if wave == 0 and (hp, nf) == (0, 0):
                                for j in range(2):
                                    nc.tensor.matmul(
                                        ps[(hp, nf)][j][:], w_sl,
                                        xf[:, ck * N + j * 256: ck * N + (j + 1) * 256],
                                        start=(ck == 0), stop=False)
                            else:
                                nc.tensor.matmul(
                                    ps[(hp, nf)][:], w_sl,
                                    xf[:, ck * N + nf * 512: ck * N + (nf + 1) * 512],
                                    start=(ck == 0), stop=False)
                    for tp in range(3):
                        for hp, nf in pairs:
                            if wave == 0 and (hp, nf) == (0, 0):
                                for j in range(2):
                                    nc.tensor.matmul(
                                        ps[(hp, nf)][j][:],
                                        wql3[:, 2 * tp:2 * tp + 2, hp * 128:(hp + 1) * 128],
                                        xfl3[:, 2 * tp:2 * tp + 2, j * 256:(j + 1) * 256],
                                        start=False, stop=(tp == 2), perf_mode=DR)
                            else:
                                nc.tensor.matmul(
                                    ps[(hp, nf)][:],
                                    wql3[:, 2 * tp:2 * tp + 2, hp * 128:(hp + 1) * 128],
                                    xfl3[:, 2 * tp:2 * tp + 2, nf * 512:(nf + 1) * 512],
                                    start=False, stop=(tp == 2), perf_mode=DR)
                    for hp, nf in pairs:
                        p = ps[(hp, nf)]
                        halves = ([(p[0], slice(0, 256)), (p[1], slice(256, 512))]
                                  if isinstance(p, list) else
                                  [(p, slice(nf * 512, (nf + 1) * 512))])
                        # full-height single-op epilogues (engine cost depends
                        # only on free size, so partition splits would double
                        # it): qTh = fp16(psum), qTl = psum - qTh
                        for pt, ns in halves:
                            nc.scalar.activation(qTh[hp][:, ns], pt[:, :],
                                                 ACT.Identity, bias=0.0, scale=1.0)
                            nc.vector.tensor_tensor(qTl[hp][:, ns], pt[:, :],
                                                    qTh[hp][:, ns], AOT.subtract)
                    for hp in range(wave * 3, wave * 3 + 3):
                        # assemble per-head packed tiles (hi on one half, lo on
                        # the other) with SBUF->SBUF DMAs on idle queues; P4
                        # consumes qp much later so latency is free
                        q_dma = nc.gpsimd.dma_start if hp % 2 == 0 else nc.sync.dma_start
                        q_dma(qp[2 * hp][0:64, :], qTh[hp][0:64, :])
                        q_dma(qp[2 * hp][64:128, :], qTl[hp][0:64, :])
                        q_dma(qp[2 * hp + 1][64:128, :], qTh[hp][64:128, :])
                        q_dma(qp[2 * hp + 1][0:64, :], qTl[hp][64:128, :])

                if upto < 3:
                    nc.gpsimd.dma_start(out_d[0:128, 0:N], qTh[0][:, 0:N])
                    continue
                # ---- phase 3: k|v chunks + spikes (fp16) + M accumulation ----
                m_ps = mps.tile([128, 6 * D], F32, name=f"m_ps{_r}", tag="m_ps")
                # software-pipelined: M-matmuls for chunk nk-1 are emitted
                # after chunk nk's psums so the PE never waits on the DVE
                # spike that produces kvs


                def emit_m(nk):
                    # m_ps packs heads 0-5 on partitions 0:64 and heads 6-11
                    # on 64:128 (tile_position col base), one PSUM bank total
                    kvs = kvss[nk]
                    for h in range(H):
                        lo, hi = (0, 64) if h < 6 else (64, 128)
                        hc = h % 6
                        nc.tensor.matmul(m_ps[lo:hi, hc * 64:(hc + 1) * 64],
                                         kvs[:, h * 64:(h + 1) * 64],
                                         kvs[:, C + h * 64: C + (h + 1) * 64],
                                         start=(nk == 0 and h in (0, 6)),
                                         stop=(nk == 7 and h in (5, 11)),
                                         tile_position=(0, lo),
                                         skip_group_check=True)

                for nk in range(8):
                    if kvss[nk] is None:
                        kvss[nk] = work.tile([128, 2 * C], FP16, name=f"kvs{nk}_{_r}", tag="kvs", bufs=3)
                    for kvf in range(3):
                        if (nk, kvf) not in kv_done:
                            emit_kv_psum(nk, kvf)
                    if nk > 1:
                        emit_m(nk - 2)
                emit_m(6)
                emit_m(7)

                if upto < 4:
                    mdump = work.tile([128, 6 * D], F32, name=f"mdump{_r}", tag="mdump")
                    nc.vector.tensor_copy(mdump[:], m_ps[:])
                    nc.sync.dma_start(out_d[0:128, 0:6 * D], mdump[:, :])
                    continue
                # ---- phase 4: y-matmul (fp16 hi+lo) -> spike -> zT (head
                # pairs packed via tile_position quadrants) ----
                m16 = const.tile([128, H * D], FP16, name=f"m16_{_r}", tag="m16")
                # copy+partition-shift: m16 holds every head's M on both
                # partition halves (sources from the packed m_ps halves).
                # Pieces ordered by first use: heads 0-1 feed the first zp
                # group, so their copy+dup chain is shortest.
                for lo_, hi_ in ((0, 128), (128, 384)):
                    cs = slice(lo_, hi_)
                    nc.scalar.activation(m16[0:64, cs], m_ps[0:64, cs], ACT.Identity,
                                         bias=0.0, scale=1.0)
                    nc.sync.dma_start(m16[64:128, cs], m16[0:64, cs])
                nc.scalar.activation(m16[64:128, 384:768], m_ps[64:128, 0:384],
                                     ACT.Identity, bias=0.0, scale=1.0)
                nc.sync.dma_start(m16[0:64, 384:768], m16[64:128, 384:768])
                z8 = const.tile([128, 6 * N], FP8, name=f"z8_{_r}", tag="z8")
                z83 = z8[:, :].rearrange("p (t n) -> p t n", t=6)
                for hp in range(6):
                    hA, hB = 2 * hp, 2 * hp + 1
                    for half in range(2):
                        zp = mm.tile([128, 512], F32, name=f"zp{hp}_{half}_{_r}", tag="mm")
                        for q8 in range(8):
                            qb = half * 8 + q8
                            # one matmul per (head, q8) region: contraction
                            # spans all 128 partitions = hi+lo halves of qp
                            # against duplicated M halves; each region is
                            # written exactly once (skip the sim's coarse
                            # zero-region group check; HW-validated construct)
                            qA = qp[hA][:, :].rearrange("p (a b) -> p a b", b=16)[:, :, qb:qb + 1]
                            qB = qp[hB][:, :].rearrange("p (a b) -> p a b", b=16)[:, :, qb:qb + 1]
                            nc.tensor.matmul(zp[0:64, q8 * 64:(q8 + 1) * 64],
                                             qA,
                                             m16[:, hA * 64:(hA + 1) * 64],
                                             start=True, stop=True,
                                             tile_position=(0, 0),
                                             skip_group_check=True)
                            nc.tensor.matmul(zp[64:128, q8 * 64:(q8 + 1) * 64],
                                             qB,
                                             m16[:, hB * 64:(hB + 1) * 64],
                                             start=True, stop=True,
                                             tile_position=(0, 64),
                                             skip_group_check=True)
                        # z encodings per head-pair block: even hp emit
                        # sign in {-1,+1} on the Activation engine, odd hp
                        # emit {0,1} on DVE; the proj weights/bias fold the
                        # difference (even rows at SP/2 plus a colsum/2 bias)
                        if hp % 2 == 0:
                            nc.scalar.activation(
                                z83[:, hp, half * 512:(half + 1) * 512], zp[:],
                                ACT.Sign, bias=tytn[:, hp:hp + 1], scale=1.0)
                        else:
                            nc.vector.tensor_scalar(
                                z83[:, hp, half * 512:(half + 1) * 512], zp[:],
                                tyt[:, hp:hp + 1], None, AOT.is_ge)

                if upto < 5:
                    nc.gpsimd.dma_start(out_d[0:128, 0:N], qTh[0][:, 0:N])
                    continue
                # ---- phase 5 (transposed): outT[cout, n] = (64/SP) * psum
                # + pb[cout]; stationary = proj weights, moving = z8, so the
                # proj bias is a per-partition Activation bias and out ap=512 ----
                for co in range(6):
                    outs = work.tile([128, N], F32, name=f"outs{co}_{_r}", tag="outs")
                    for half in range(2):
                        pp = mm.tile([128, 512], F32, name=f"pp{co}_{half}_{_r}", tag="mm")
                        for hl, p3 in enumerate((phi3, plo3)):
                            for tp in range(3):
                                nc.tensor.matmul(
                                    pp[:],
                                    p3[:, 2 * tp:2 * tp + 2, co * 128:(co + 1) * 128],
                                    z83[:, 2 * tp:2 * tp + 2, half * 512:(half + 1) * 512],
                                    start=(hl == 0 and tp == 0),
                                    stop=(hl == 1 and tp == 2),
                                    perf_mode=DR)
                        nc.scalar.activation(outs[:, half * 512:(half + 1) * 512],
                                             pp[:], ACT.Identity,
                                             bias=pb[:, co:co + 1], scale=1.0 / SP)
                        if co == 5:
                            # split the last chunk's writes across both DMA
                            # queues so the final drain halves
                            for qi, q_dma in enumerate((nc.gpsimd.dma_start,
                                                        nc.sync.dma_start)):
                                q_dma(out_d[co * 128:(co + 1) * 128,
                                            half * 512 + qi * 256:half * 512 + (qi + 1) * 256],
                                      outs[:, half * 512 + qi * 256:half * 512 + (qi + 1) * 256])
                        else:
                            o_dma = nc.gpsimd.dma_start if half == 0 else nc.sync.dma_start
                            o_dma(out_d[co * 128:(co + 1) * 128,
                                        half * 512:(half + 1) * 512],
                                  outs[:, half * 512:(half + 1) * 512])

    return nc


def prep_params(inputs):
    """Host-side folding of BN/Hoyer params + weight transposes/splits."""
    d = {k: np.asarray(v, np.float32) for k, v in inputs.items()}

    def fold(p, a):
        s = d[p + '_g'] / np.sqrt(d[p + '_v'] + EPS)
        thr = float(d[a + '_thr'])
        A = s / thr
        Bc = (d[p + '_b'] - d[p + '_m'] * s) / thr
        T2 = XS * d[a + '_run'] - Bc
        return A.astype(np.float32), T2.astype(np.float32)

    A_x, T2_x = fold('n', 'a')
    A_k, T2_k = fold('nk', 'ak')
    A_v, T2_v = fold('nv', 'av')
    A_o, T2_o = fold('no', 'ao')

    Wt = d['qkv_w'].T.copy()                       # [C, 3C]
    colscale = np.concatenate([np.repeat(A_o, D),
                               np.repeat(A_k, D), np.repeat(A_v, D)])
    Wt *= colscale[None, :]
    wq = np.ascontiguousarray(Wt[:, 0:C])
    wkv = np.ascontiguousarray(Wt[:, C:3 * C])
    wqh = wq.astype(np.float16)
    wql = ((wq - wqh.astype(np.float32)) * 128).astype(E5)
    wkvh = wkv.astype(np.float16)
    wkvl = ((wkv - wkvh.astype(np.float32)) * 128).astype(E5)

    Pt = np.ascontiguousarray(d['proj_w'].T)       # [C, C]
    rows_even = (np.arange(C) // 128) % 2 == 0
    rowscale = np.where(rows_even, SP / 2.0, SP).astype(np.float32)
    Pt8 = Pt * rowscale[:, None]
    p_hi = Pt8.astype(E4)
    p_lo = (Pt8 - p_hi.astype(np.float32)).astype(E4)
    colsum_even = Pt[rows_even, :].sum(axis=0)

    def part6(vec):  # [768] -> [128, 6]; col ck = partition chunk ck
        return np.ascontiguousarray(vec.reshape(6, 128).T)

    return dict(
        wqh=wqh, wql=wql, wkvh=wkvh, wkvl=wkvl, p_hi=p_hi, p_lo=p_lo,
        txA=part6(np.repeat(A_x, D)), txT=part6(np.repeat(T2_x, D)),
        tkv=np.ascontiguousarray(np.broadcast_to(
            np.concatenate([np.repeat(T2_k, D), np.repeat(T2_v, D)]),
            (128, 2 * C))).astype(np.float32),
        tyT=part6(np.repeat(T2_o, D)),
        pb=part6(d['proj_b'] + 0.5 * colsum_even),
        tytn=part6(-np.repeat(T2_o, D)),
    )


def make_in_maps(inputs):
    shared = prep_params(inputs)
    x = np.asarray(inputs['x'], np.float32)
    return [dict(shared, xt=np.ascontiguousarray(x[c].T)) for c in range(NCORES)]


_CACHE = {}


def _make_executor(nc, n_cores=NCORES):
    """Jitted SPMD executor for the Bass graph (mirrors
    concourse.bass2jax.run_bass_via_pjrt, kept reusable for repeat runs)."""
    import jax
    from jax.sharding import Mesh, PartitionSpec
    from jax.experimental.shard_map import shard_map
    from concourse.bass2jax import (_bass_exec_p, install_neuronx_cc_hook,
                                    partition_id_tensor)
    install_neuronx_cc_hook()
    partition_name = nc.partition_id_tensor.name if nc.partition_id_tensor else None
    in_names, out_names, out_avals, zero_outs = [], [], [], []
    for alloc in nc.m.functions[0].allocations:
        if not isinstance(alloc, mybir.MemoryLocationSet):
            continue
        name = alloc.memorylocations[0].name
        if alloc.kind == "ExternalInput":
            if name != partition_name:
                in_names.append(name)
        elif alloc.kind == "ExternalOutput":
            out_names.append(name)
            shape = tuple(alloc.tensor_shape)
            dtype = mybir.dt.np(alloc.dtype)
            out_avals.append(jax.core.ShapedArray(shape, dtype))
            zero_outs.append(np.zeros(shape, dtype))
    n_params = len(in_names)
    n_outs = len(out_avals)
    all_in_names = list(in_names) + list(out_names)
    if partition_name is not None:
        all_in_names.append(partition_name)

    def _body(*args):
        operands = list(args)
        if partition_name is not None:
            operands.append(partition_id_tensor())
        outs = _bass_exec_p.bind(
            *operands,
            out_avals=tuple(out_avals), in_names=tuple(all_in_names),
            out_names=tuple(out_names), lowering_input_output_aliases=(),
            sim_require_finite=True, sim_require_nnan=True, nc=nc,
        )
        return tuple(outs)

    try:
        devices = jax.devices("axon")[:n_cores]
    except RuntimeError:
        devices = jax.devices()[:n_cores]
    mesh = Mesh(np.asarray(devices), ("core",))
    in_specs = (PartitionSpec("core"),) * (n_params + n_outs)
    out_specs = (PartitionSpec("core"),) * n_outs
    donate = tuple(range(n_params, n_params + n_outs))
    sharded = jax.jit(
        shard_map(_body, mesh=mesh, in_specs=in_specs, out_specs=out_specs,
                  check_rep=False),
        donate_argnums=donate, keep_unused=True,
    )

    def run(in_maps):
        per_core = [[np.asarray(m[n]) for n in in_names] for m in in_maps]
        concat_in = [np.concatenate([per_core[c][i] for c in range(n_cores)], axis=0)
                     for i in range(n_params)]
        concat_zeros = [np.zeros((n_cores * z.shape[0], *z.shape[1:]), z.dtype)
                        for z in zero_outs]
        out_arrs = sharded(*concat_in, *concat_zeros)
        return [
            {name: np.asarray(out_arrs[i]).reshape(n_cores, *out_avals[i].shape)[c]
             for i, name in enumerate(out_names)}
            for c in range(n_cores)
        ], out_arrs

    def run_device_args(concat_in, concat_zeros):
        return sharded(*concat_in, *concat_zeros)

    return run, run_device_args, (in_names, out_names, out_avals, zero_outs, n_params)


def kernel(**inputs) -> np.ndarray:
    if 'exec' not in _CACHE:
        nc = build_nc()
        nc.compile()
        run, run_dev, meta = _make_executor(nc, NCORES)
        _CACHE['exec'] = (nc, run, run_dev, meta)
    nc, run, run_dev, meta = _CACHE['exec']
    in_maps = make_in_maps(inputs)
    results, _ = run(in_maps)
    return np.stack([np.ascontiguousarray(results[c]['out'].T)
                     for c in range(NCORES)]).astype(np.float32)

